# revision 1
# baseline (speedup 1.0000x reference)
"""BinaryDense kernel for Trainium2: out = sign(x) @ sign(w).

x: [8192, 2048] f32, w: [2048, 2048] f32 -> out: [8192, 2048] f32.

Strategy: data-parallel shard of the batch dim across 8 NeuronCores
(1024 rows each, w replicated). Per core:
  - w streamed in by n-column "passes", binarized on DVE to fp8e4 (+-0.5,
    via one tensor_scalar: (w >= 0) - 0.5), kept resident in SBUF (4MB).
  - x loaded by 128-row m-tiles, transposed 128x128-blockwise on the PE
    (matmul-with-identity), binarized+evicted PSUM->SBUF in fused DVE ops
    to an fp8 x^T resident [128, KT, B] laid out for matmul lhsT slices.
  - Matmuls in fp8 DoubleRow mode (K=256 per instruction), accumulating
    in PSUM fp32; n-pass-major loop so compute can start as soon as the
    first column-slice of w has landed (w DMA is the serial prefix).
  - PSUM evicted on the Scalar engine with scale=4.0 (products are
    (+-0.5)^2 = +-0.25), giving bit-exact integer f32 outputs.

All arithmetic is exact: +-0.5 is exact in fp8e4, products +-0.25 exact,
sums are multiples of 0.25 bounded by 512 (fp32-exact), x4 exact.
"""

import sys

if "/opt/trn_rl_repo" not in sys.path:
    sys.path.insert(0, "/opt/trn_rl_repo")

import numpy as np

B_FULL, D_IN, UNITS = 8192, 2048, 2048
N_CORES = 8
B_CORE = B_FULL // N_CORES  # 1024
P = 128


def build_kernel(B=B_CORE, D=D_IN, U=UNITS, pass_w=512, use_dr=True):
    """Build (and compile) the per-core Bass kernel. Returns the Bacc nc."""
    from concourse import bacc
    import concourse.mybir as mybir
    import concourse.tile as tile
    from concourse.masks import make_identity

    f32 = mybir.dt.float32
    f8 = mybir.dt.float8e4

    assert B % P == 0 and D % P == 0 and U % pass_w == 0 and pass_w % 512 == 0
    MT = B // P          # m-tiles (8)
    KT = D // P          # k-subtiles (16)
    NQ = U // pass_w     # n passes
    NB = pass_w // 512   # psum banks per (m-tile, pass)
    KSUB_CHUNK = 4       # k-subtiles per w DMA+binarize chunk

    nc = bacc.Bacc("TRN2", target_bir_lowering=False)
    x_d = nc.dram_tensor("x", [B, D], f32, kind="ExternalInput")
    w_d = nc.dram_tensor("w", [D, U], f32, kind="ExternalInput")
    o_d = nc.dram_tensor("out", [B, U], f32, kind="ExternalOutput")

    x_ap = x_d[:].rearrange("(j p) d -> j p d", p=P)       # [MT, 128, D]
    w_ap = w_d[:].rearrange("(s p) u -> p s u", p=P)       # [128, KT, U]
    o_ap = o_d[:].rearrange("(j p) u -> j p u", p=P)       # [MT, 128, U]

    GE = mybir.AluOpType.is_ge
    SUB = mybir.AluOpType.subtract

    with tile.TileContext(nc) as tc, \
         tc.tile_pool(name="const", bufs=1) as constp, \
         tc.tile_pool(name="wstage", bufs=2) as wstage, \
         tc.tile_pool(name="xstage", bufs=2) as xstage, \
         tc.tile_pool(name="resident", bufs=1) as resident, \
         tc.tile_pool(name="ostage", bufs=3) as ostage, \
         tc.tile_pool(name="tpsum", bufs=2, space="PSUM") as tpsum, \
         tc.tile_pool(name="mpsum", bufs=4, space="PSUM") as mpsum:

        ident = constp.tile([P, P], f32)
        make_identity(nc, ident)

        w8 = resident.tile([P, KT, U], f8)    # binarized w, k-subtile major
        xT8 = resident.tile([P, KT, B], f8)   # binarized x^T

        def emit_x_tile(j):
            xs = xstage.tile([P, D], f32, tag="xs")
            nc.sync.dma_start(xs, x_ap[j])
            for g in range(KT // 4):
                tp = tpsum.tile([P, 4 * P], f32, tag="tp")
                for s in range(4):
                    ks = 4 * g + s
                    nc.tensor.transpose(
                        tp[:, P * s:P * (s + 1)],
                        xs[:, P * ks:P * (ks + 1)],
                        ident,
                    )
                # fused binarize + PSUM->SBUF evict: (x >= 0) - 0.5
                nc.vector.tensor_scalar(
                    xT8[:, 4 * g:4 * (g + 1), P * j:P * (j + 1)],
                    tp, 0.0, 0.5, GE, SUB,
                )

        def emit_w_pass(q):
            n0 = q * pass_w
            for c in range(KT // KSUB_CHUNK):
                ws = wstage.tile([P, KSUB_CHUNK, pass_w], f32, tag="ws")
                ksl = slice(c * KSUB_CHUNK, (c + 1) * KSUB_CHUNK)
                nc.sync.dma_start(ws, w_ap[:, ksl, n0:n0 + pass_w])
                nc.vector.tensor_scalar(
                    w8[:, ksl, n0:n0 + pass_w], ws, 0.0, 0.5, GE, SUB,
                )

        def emit_mm_pass(q, j, ost):
            for b in range(NB):
                n0 = q * pass_w + b * 512
                ps = mpsum.tile([P, 512], f32, tag="ps")
                if use_dr:
                    for ks in range(0, KT, 2):
                        nc.tensor.matmul(
                            ps,
                            lhsT=xT8[:, ks:ks + 2, P * j:P * (j + 1)],
                            rhs=w8[:, ks:ks + 2, n0:n0 + 512],
                            start=(ks == 0), stop=(ks == KT - 2),
                            perf_mode=mybir.MatmulPerfMode.DoubleRow,
                        )
                else:
                    for ks in range(KT):
                        nc.tensor.matmul(
                            ps,
                            lhsT=xT8[:, ks, P * j:P * (j + 1)],
                            rhs=w8[:, ks, n0:n0 + 512],
                            start=(ks == 0), stop=(ks == KT - 1),
                        )
                # evict with x4 scale: (+-0.5 * +-0.5) sums -> integer out
                nc.scalar.activation(
                    ost[:, 512 * b:512 * (b + 1)], ps,
                    mybir.ActivationFunctionType.Copy, scale=4.0,
                )

        # Emission schedule: x[0..1] first (PE transposes can start),
        # then the first w pass-slice, remaining x tiles, remaining w
        # slices, then all matmul passes.
        emit_x_tile(0)
        emit_x_tile(1)
        emit_w_pass(0)
        for j in range(2, MT):
            emit_x_tile(j)
        for q in range(1, NQ):
            emit_w_pass(q)
        for q in range(NQ):
            for j in range(MT):
                ost = ostage.tile([P, pass_w], f32, tag="ost")
                emit_mm_pass(q, j, ost)
                nc.sync.dma_start(o_ap[j, :, q * pass_w:(q + 1) * pass_w], ost)

    nc.compile()
    return nc


_NC_CACHE = {}
LAST_RESULTS = {}


def _get_nc(**kwargs):
    key = tuple(sorted(kwargs.items()))
    if key not in _NC_CACHE:
        _NC_CACHE[key] = build_kernel(**kwargs)
    return _NC_CACHE[key]


def kernel(x, w, _trace=False, _trace_cores=None, **build_kwargs):
    from concourse.bass_utils import run_bass_kernel_spmd

    x = np.asarray(x, dtype=np.float32)
    w = np.asarray(w, dtype=np.float32)
    assert x.shape == (B_FULL, D_IN) and w.shape == (D_IN, UNITS)

    nc = _get_nc(**build_kwargs)
    in_maps = [
        {"x": np.ascontiguousarray(x[c * B_CORE:(c + 1) * B_CORE]), "w": w}
        for c in range(N_CORES)
    ]
    br = run_bass_kernel_spmd(
        nc, in_maps, list(range(N_CORES)),
        trace=_trace, trace_cores=_trace_cores,
    )
    LAST_RESULTS["br"] = br
    out = np.concatenate(
        [br.results[c]["out"] for c in range(N_CORES)], axis=0
    )
    return out


if __name__ == "__main__":
    rng = np.random.default_rng(0)
    x = rng.standard_normal((B_FULL, D_IN), dtype=np.float32)
    w = (rng.standard_normal((D_IN, UNITS), dtype=np.float32) * 0.1).astype(
        np.float32
    )
    out = kernel(x, w)
    exp = np.sign(x + (x == 0)) @ np.sign(w + (w == 0))
    print("max abs err:", np.max(np.abs(out - exp)))


# revision 3
# speedup vs baseline: 1.1361x; 1.1361x over previous
"""BinaryDense kernel for Trainium2: out = sign(x) @ sign(w).

x: [8192, 2048] f32, w: [2048, 2048] f32 -> out: [8192, 2048] f32.

Strategy: data-parallel shard of the batch dim across 8 NeuronCores
(1024 rows each, w replicated). Per core:
  - w streamed in column-"pass" slices, sub-tiled along k so matmuls can
    start before the whole slice lands; binarized on DVE to fp8e4 (+-0.5
    via one tensor_scalar: (w >= 0) - 0.5) into per-(pass, k-half)
    resident SBUF tiles (separate tiles so Tile's dependency tracking
    doesn't serialize passes against later binarize writes).
  - x loaded by 128-row m-tiles, transposed 128x128-blockwise on the PE
    (matmul-with-identity), binarized+evicted PSUM->SBUF in fused DVE
    ops into per-m-tile fp8 x^T residents.
  - Matmuls in fp8 DoubleRow mode (K=256 per instruction), accumulating
    in PSUM fp32, n-pass-major so compute overlaps the w DMA stream.
  - PSUM evicted on the Scalar engine with scale=4.0 (products are
    (+-0.5)^2 = +-0.25), giving bit-exact integer f32 outputs.

All arithmetic is exact: +-0.5 exact in fp8e4, products +-0.25 exact,
sums are multiples of 0.25 bounded by 512 (fp32-exact), x4 exact.
"""

import sys

if "/opt/trn_rl_repo" not in sys.path:
    sys.path.insert(0, "/opt/trn_rl_repo")

import numpy as np

B_FULL, D_IN, UNITS = 8192, 2048, 2048
N_CORES = 8
B_CORE = B_FULL // N_CORES  # 1024
P = 128


def build_kernel(B=B_CORE, D=D_IN, U=UNITS, pass_w=1024, k_halves=2,
                 ksub_chunk=4, use_dr=True):
    """Build (and compile) the per-core Bass kernel. Returns the Bacc nc."""
    from concourse import bacc
    import concourse.mybir as mybir
    import concourse.tile as tile
    from concourse.masks import make_identity

    f32 = mybir.dt.float32
    f8 = mybir.dt.float8e4

    assert B % P == 0 and D % P == 0 and U % pass_w == 0 and pass_w % 512 == 0
    MT = B // P            # m-tiles (8)
    KT = D // P            # k-subtiles (16)
    NQ = U // pass_w       # n passes (2)
    NB = pass_w // 512     # psum banks per (m-tile, pass) (2)
    KH = KT // k_halves    # k-subtiles per w sub-tile (8)
    assert KH % ksub_chunk == 0
    if use_dr:
        assert KH % 2 == 0

    nc = bacc.Bacc("TRN2", target_bir_lowering=False)
    x_d = nc.dram_tensor("x", [B, D], f32, kind="ExternalInput")
    w_d = nc.dram_tensor("w", [D, U], f32, kind="ExternalInput")
    o_d = nc.dram_tensor("out", [B, U], f32, kind="ExternalOutput")

    x_ap = x_d[:].rearrange("(j p) d -> j p d", p=P)       # [MT, 128, D]
    w_ap = w_d[:].rearrange("(s p) u -> p s u", p=P)       # [128, KT, U]
    o_ap = o_d[:].rearrange("(j p) u -> j p u", p=P)       # [MT, 128, U]

    GE = mybir.AluOpType.is_ge
    SUB = mybir.AluOpType.subtract

    with tile.TileContext(nc) as tc, \
         tc.tile_pool(name="const", bufs=1) as constp, \
         tc.tile_pool(name="wstage", bufs=2) as wstage, \
         tc.tile_pool(name="xstage", bufs=2) as xstage, \
         tc.tile_pool(name="resident", bufs=1) as resident, \
         tc.tile_pool(name="ostage", bufs=3) as ostage, \
         tc.tile_pool(name="tpsum", bufs=2, space="PSUM") as tpsum, \
         tc.tile_pool(name="mpsum", bufs=4, space="PSUM") as mpsum:

        ident = constp.tile([P, P], f32)
        make_identity(nc, ident)

        # separate resident tiles => fine-grained scheduler dependencies
        w8 = [[resident.tile([P, KH, pass_w], f8, name=f"w8_{q}_{h}")
               for h in range(k_halves)] for q in range(NQ)]
        xT8 = [resident.tile([P, KT, P], f8, name=f"xT8_{j}")
               for j in range(MT)]

        def emit_x_tile(j):
            xs = xstage.tile([P, D], f32, tag="xs")
            nc.sync.dma_start(xs, x_ap[j])
            for g in range(KT // 4):
                tp = tpsum.tile([P, 4 * P], f32, tag="tp")
                for s in range(4):
                    ks = 4 * g + s
                    nc.tensor.transpose(
                        tp[:, P * s:P * (s + 1)],
                        xs[:, P * ks:P * (ks + 1)],
                        ident,
                    )
                # fused binarize + PSUM->SBUF evict: (x >= 0) - 0.5
                nc.vector.tensor_scalar(
                    xT8[j][:, 4 * g:4 * (g + 1), :],
                    tp, 0.0, 0.5, GE, SUB,
                )

        def emit_w_subtile(q, h):
            n0 = q * pass_w
            for c in range(KH // ksub_chunk):
                ws = wstage.tile([P, ksub_chunk, pass_w], f32, tag="ws")
                ks0 = h * KH + c * ksub_chunk
                nc.sync.dma_start(
                    ws, w_ap[:, ks0:ks0 + ksub_chunk, n0:n0 + pass_w])
                nc.vector.tensor_scalar(
                    w8[q][h][:, c * ksub_chunk:(c + 1) * ksub_chunk, :],
                    ws, 0.0, 0.5, GE, SUB,
                )

        def emit_mm(q, j):
            ost = ostage.tile([P, pass_w], f32, tag="ost")
            pss = [mpsum.tile([P, 512], f32, tag="ps", name=f"ps_{q}_{j}_{b}")
               for b in range(NB)]
            step = 2 if use_dr else 1
            for h in range(k_halves):
                for kc in range(0, KH, step):
                    ks = h * KH + kc
                    first = ks == 0
                    last = ks + step >= KT
                    for b in range(NB):
                        if use_dr:
                            nc.tensor.matmul(
                                pss[b],
                                lhsT=xT8[j][:, ks:ks + 2, :],
                                rhs=w8[q][h][:, kc:kc + 2, 512 * b:512 * (b + 1)],
                                start=first, stop=last,
                                perf_mode=mybir.MatmulPerfMode.DoubleRow,
                            )
                        else:
                            nc.tensor.matmul(
                                pss[b],
                                lhsT=xT8[j][:, ks, :],
                                rhs=w8[q][h][:, kc, 512 * b:512 * (b + 1)],
                                start=first, stop=last,
                            )
            for b in range(NB):
                # evict with x4 scale: (+-0.5 * +-0.5) sums -> integer out
                nc.scalar.activation(
                    ost[:, 512 * b:512 * (b + 1)], pss[b],
                    mybir.ActivationFunctionType.Copy, scale=4.0,
                )
            nc.sync.dma_start(o_ap[j, :, q * pass_w:(q + 1) * pass_w], ost)

        # Emission schedule (defines DMA queue order and per-engine
        # instruction order): x0/x1 first so PE transposes start early,
        # then the first w pass (all its k sub-tiles), then pass-0
        # matmuls interleaved with remaining x tiles, then later passes.
        emit_x_tile(0)
        emit_x_tile(1)
        for h in range(k_halves):
            emit_w_subtile(0, h)
        for j in range(2, MT):
            emit_x_tile(j)
            emit_mm(0, j - 2)
        for q in range(1, NQ):
            for h in range(k_halves):
                emit_w_subtile(q, h)
        emit_mm(0, MT - 2)
        emit_mm(0, MT - 1)
        for q in range(1, NQ):
            for j in range(MT):
                emit_mm(q, j)

    nc.compile()
    return nc


_NC_CACHE = {}
LAST_RESULTS = {}


def _get_nc(**kwargs):
    key = tuple(sorted(kwargs.items()))
    if key not in _NC_CACHE:
        _NC_CACHE[key] = build_kernel(**kwargs)
    return _NC_CACHE[key]


def kernel(x, w, _trace=False, _trace_cores=None, **build_kwargs):
    from concourse.bass_utils import run_bass_kernel_spmd

    x = np.asarray(x, dtype=np.float32)
    w = np.asarray(w, dtype=np.float32)
    assert x.shape == (B_FULL, D_IN) and w.shape == (D_IN, UNITS)

    nc = _get_nc(**build_kwargs)
    in_maps = [
        {"x": np.ascontiguousarray(x[c * B_CORE:(c + 1) * B_CORE]), "w": w}
        for c in range(N_CORES)
    ]
    br = run_bass_kernel_spmd(
        nc, in_maps, list(range(N_CORES)),
        trace=_trace, trace_cores=_trace_cores,
    )
    LAST_RESULTS["br"] = br
    out = np.concatenate(
        [br.results[c]["out"] for c in range(N_CORES)], axis=0
    )
    return out


if __name__ == "__main__":
    rng = np.random.default_rng(0)
    x = rng.standard_normal((B_FULL, D_IN), dtype=np.float32)
    w = (rng.standard_normal((D_IN, UNITS), dtype=np.float32) * 0.1).astype(
        np.float32
    )
    out = kernel(x, w)
    exp = np.sign(x + (x == 0)) @ np.sign(w + (w == 0))
    print("max abs err:", np.max(np.abs(out - exp)))


# revision 5
# speedup vs baseline: 1.2260x; 1.0791x over previous
"""BinaryDense kernel for Trainium2: out = sign(x) @ sign(w).

x: [8192, 2048] f32, w: [2048, 2048] f32 -> out: [8192, 2048] f32.

Strategy: data-parallel shard of the batch dim across 8 NeuronCores
(1024 rows each, w replicated). The host hands each core its x shard
pre-transposed (pure layout choice for the shard), so the contraction
dim lands on SBUF partitions with no on-chip transpose. Per core:
  - w streamed in (pass, k-sub-tile) slices of [128, 4ksub, 512cols]
    (1MB granule), binarized on DVE to fp8e4 (+-0.5 via one
    tensor_scalar: (w >= 0) - 0.5) into separate small resident tiles
    (separate tiles => fine-grained scheduler dependencies, so matmuls
    start as soon as their slice has landed).
  - x^T loaded in m-pair column groups [128, 16ksub, 256], binarized the
    same way into fp8 residents.
  - Matmuls in fp8 DoubleRow mode (K=256 per instruction, N=512),
    accumulating in PSUM fp32, n-pass-major (4 passes of 512 columns) so
    compute overlaps the w DMA stream.
  - PSUM evicted on the Scalar engine with scale=4.0 (products are
    (+-0.5)^2 = +-0.25) to fp16 output tiles (integers <= 2048 are
    exact in fp16), halving the store traffic; host widens to f32.

All arithmetic is exact: +-0.5 exact in fp8e4, products +-0.25 exact,
sums are multiples of 0.25 bounded by 512 (fp32-exact), x4 exact,
results are integers in [-2048, 2048], all exactly representable in
fp16. The host fp16->f32 widening is exact.
"""

import sys

if "/opt/trn_rl_repo" not in sys.path:
    sys.path.insert(0, "/opt/trn_rl_repo")

import numpy as np

B_FULL, D_IN, UNITS = 8192, 2048, 2048
N_CORES = 8
B_CORE = B_FULL // N_CORES  # 1024
P = 128


def build_kernel(B=B_CORE, D=D_IN, U=UNITS, pass_w=512, wsub=4, xgrp=2,
                 use_dr=True, out_dt="float16"):
    """Build (and compile) the per-core Bass kernel. Returns the Bacc nc.

    pass_w: output columns per pass (multiple of 512)
    wsub:   k-subtiles per resident w sub-tile (DMA/binarize granule)
    xgrp:   m-tiles (128 cols of x^T) per resident x^T group
    """
    from concourse import bacc
    import concourse.mybir as mybir
    import concourse.tile as tile

    f32 = mybir.dt.float32
    f8 = mybir.dt.float8e4
    odt = getattr(mybir.dt, out_dt)

    assert B % P == 0 and D % P == 0 and U % pass_w == 0 and pass_w % 512 == 0
    MT = B // P            # m-tiles (8)
    KT = D // P            # k-subtiles (16)
    NQ = U // pass_w       # n passes (4)
    NB = pass_w // 512     # psum banks per (m-tile, pass) (1)
    KS = KT // wsub        # w sub-tiles per pass (4)
    XG = MT // xgrp        # x^T groups (4)
    step = 2 if use_dr else 1
    assert wsub % step == 0

    nc = bacc.Bacc("TRN2", target_bir_lowering=False)
    x_d = nc.dram_tensor("xT", [D, B], f32, kind="ExternalInput")
    w_d = nc.dram_tensor("w", [D, U], f32, kind="ExternalInput")
    o_d = nc.dram_tensor("out", [B, U], odt, kind="ExternalOutput")

    x_ap = x_d[:].rearrange("(s p) m -> p s m", p=P)       # [128, KT, B]
    w_ap = w_d[:].rearrange("(s p) u -> p s u", p=P)       # [128, KT, U]
    o_ap = o_d[:].rearrange("(j p) u -> j p u", p=P)       # [MT, 128, U]

    GE = mybir.AluOpType.is_ge
    SUB = mybir.AluOpType.subtract

    with tile.TileContext(nc) as tc, \
         tc.tile_pool(name="wstage", bufs=3) as wstage, \
         tc.tile_pool(name="xstage", bufs=2) as xstage, \
         tc.tile_pool(name="resident", bufs=1) as resident, \
         tc.tile_pool(name="mpsum", bufs=8, space="PSUM") as mpsum:

        # separate resident tiles => fine-grained scheduler dependencies
        w8 = [[resident.tile([P, wsub, pass_w], f8, name=f"w8_{q}_{h}")
               for h in range(KS)] for q in range(NQ)]
        xT8 = [resident.tile([P, KT, xgrp * P], f8, name=f"xT8_{g}")
               for g in range(XG)]
        ost = [resident.tile([P, U], odt, name=f"ost_{j}")
               for j in range(MT)]

        def emit_x_group(g):
            m0 = g * xgrp * P
            xs = xstage.tile([P, KT, xgrp * P], f32, tag="xs")
            nc.sync.dma_start(xs, x_ap[:, :, m0:m0 + xgrp * P])
            nc.vector.tensor_scalar(xT8[g], xs, 0.0, 0.5, GE, SUB)

        def emit_w_subtile(q, h):
            n0 = q * pass_w
            ws = wstage.tile([P, wsub, pass_w], f32, tag="ws")
            ks0 = h * wsub
            nc.sync.dma_start(ws, w_ap[:, ks0:ks0 + wsub, n0:n0 + pass_w])
            nc.vector.tensor_scalar(w8[q][h], ws, 0.0, 0.5, GE, SUB)

        def emit_mm(q, j):
            g, jo = j // xgrp, (j % xgrp) * P
            pss = [mpsum.tile([P, 512], f32, tag="ps", name=f"ps_{q}_{j}_{b}")
                   for b in range(NB)]
            for h in range(KS):
                for kc in range(0, wsub, step):
                    ks = h * wsub + kc
                    first = ks == 0
                    last = ks + step >= KT
                    for b in range(NB):
                        if use_dr:
                            nc.tensor.matmul(
                                pss[b],
                                lhsT=xT8[g][:, ks:ks + 2, jo:jo + P],
                                rhs=w8[q][h][:, kc:kc + 2, 512 * b:512 * (b + 1)],
                                start=first, stop=last,
                                perf_mode=mybir.MatmulPerfMode.DoubleRow,
                            )
                        else:
                            nc.tensor.matmul(
                                pss[b],
                                lhsT=xT8[g][:, ks, jo:jo + P],
                                rhs=w8[q][h][:, kc, 512 * b:512 * (b + 1)],
                                start=first, stop=last,
                            )
            for b in range(NB):
                # evict with x4 scale: (+-0.5 * +-0.5) sums -> integer out
                nc.scalar.activation(
                    ost[j][:, q * pass_w + 512 * b:q * pass_w + 512 * (b + 1)],
                    pss[b], mybir.ActivationFunctionType.Copy, scale=4.0,
                )

        def emit_store(j):
            nc.sync.dma_start(o_ap[j], ost[j])

        # Emission schedule (defines DMA queue order and per-engine
        # instruction order). Interleave w pass-slices, x groups and
        # matmuls so PE work is available as data lands.
        if (NQ, KS, XG, MT) == (4, 4, 4, 8):
            emit_w_subtile(0, 0)
            emit_w_subtile(0, 1)
            emit_x_group(0)
            emit_w_subtile(0, 2)
            emit_w_subtile(0, 3)
            emit_x_group(1)
            emit_mm(0, 0)
            emit_mm(0, 1)
            for h in range(KS):
                emit_w_subtile(1, h)
            emit_mm(0, 2)
            emit_mm(0, 3)
            emit_x_group(2)
            emit_mm(1, 0)
            emit_mm(1, 1)
            emit_x_group(3)
            emit_mm(0, 4)
            emit_mm(0, 5)
            emit_mm(1, 2)
            emit_mm(1, 3)
            for h in range(KS):
                emit_w_subtile(2, h)
            emit_mm(0, 6)
            emit_mm(0, 7)
            for j in range(4, MT):
                emit_mm(1, j)
            for h in range(KS):
                emit_w_subtile(3, h)
            for j in range(MT):
                emit_mm(2, j)
            for j in range(MT):
                emit_mm(3, j)
                emit_store(j)
        else:
            # generic fallback (used by small-shape tests)
            for q in range(NQ):
                for h in range(KS):
                    emit_w_subtile(q, h)
            for g in range(XG):
                emit_x_group(g)
            for q in range(NQ):
                for j in range(MT):
                    emit_mm(q, j)
                    if q == NQ - 1:
                        emit_store(j)

    nc.compile()
    return nc


_NC_CACHE = {}
LAST_RESULTS = {}


def _get_nc(**kwargs):
    key = tuple(sorted(kwargs.items()))
    if key not in _NC_CACHE:
        _NC_CACHE[key] = build_kernel(**kwargs)
    return _NC_CACHE[key]


def kernel(x, w, _trace=False, _trace_cores=None, **build_kwargs):
    from concourse.bass_utils import run_bass_kernel_spmd

    x = np.asarray(x, dtype=np.float32)
    w = np.asarray(w, dtype=np.float32)
    assert x.shape == (B_FULL, D_IN) and w.shape == (D_IN, UNITS)

    nc = _get_nc(**build_kwargs)
    in_maps = [
        {"xT": np.ascontiguousarray(x[c * B_CORE:(c + 1) * B_CORE].T),
         "w": w}
        for c in range(N_CORES)
    ]
    br = run_bass_kernel_spmd(
        nc, in_maps, list(range(N_CORES)),
        trace=_trace, trace_cores=_trace_cores,
    )
    LAST_RESULTS["br"] = br
    out = np.concatenate(
        [br.results[c]["out"].astype(np.float32) for c in range(N_CORES)],
        axis=0,
    )
    return out


if __name__ == "__main__":
    rng = np.random.default_rng(0)
    x = rng.standard_normal((B_FULL, D_IN), dtype=np.float32)
    w = (rng.standard_normal((D_IN, UNITS), dtype=np.float32) * 0.1).astype(
        np.float32
    )
    out = kernel(x, w)
    exp = np.sign(x + (x == 0)) @ np.sign(w + (w == 0))
    print("max abs err:", np.max(np.abs(out - exp)))


# revision 7
# speedup vs baseline: 1.3638x; 1.1124x over previous
"""BinaryDense kernel for Trainium2: out = sign(x) @ sign(w).

x: [8192, 2048] f32, w: [2048, 2048] f32 -> out: [8192, 2048] f32.

Strategy: data-parallel shard of the batch dim across 8 NeuronCores
(1024 rows each, w replicated). The host hands each core its x shard
pre-transposed (pure layout choice for the shard), so the contraction
dim lands on SBUF partitions with no on-chip transpose. Per core:
  - w streamed in (pass, k-sub-tile) slices of [128, 4ksub, 512cols]
    (1MB granule), binarized on DVE to fp8e4 (+-0.5 via one
    tensor_scalar: (w >= 0) - 0.5) into separate small resident tiles
    (separate tiles => fine-grained scheduler dependencies, so matmuls
    start as soon as their slice has landed).
  - x^T loaded in m-pair column groups [128, 16ksub, 256], binarized the
    same way into fp8 residents.
  - Matmuls in fp8 DoubleRow mode (K=256 per instruction, N=512),
    accumulating in PSUM fp32, n-pass-major (4 passes of 512 columns) so
    compute overlaps the w DMA stream.
  - PSUM evicted on the Scalar engine with scale=4.0 (products are
    (+-0.5)^2 = +-0.25) to fp16 output tiles (integers <= 2048 are
    exact in fp16), halving the store traffic; host widens to f32.

All arithmetic is exact: +-0.5 exact in fp8e4, products +-0.25 exact,
sums are multiples of 0.25 bounded by 512 (fp32-exact), x4 exact,
results are integers in [-2048, 2048], all exactly representable in
fp16. The host fp16->f32 widening is exact.
"""

import sys

if "/opt/trn_rl_repo" not in sys.path:
    sys.path.insert(0, "/opt/trn_rl_repo")

import numpy as np

B_FULL, D_IN, UNITS = 8192, 2048, 2048
N_CORES = 8
B_CORE = B_FULL // N_CORES  # 1024
P = 128


def build_kernel(B=B_CORE, D=D_IN, U=UNITS, pass_w=512, wsub=4, xgrp=2,
                 use_dr=True, out_dt="float16"):
    """Build (and compile) the per-core Bass kernel. Returns the Bacc nc.

    pass_w: output columns per pass (multiple of 512)
    wsub:   k-subtiles per resident w sub-tile (DMA/binarize granule)
    xgrp:   m-tiles (128 cols of x^T) per resident x^T group
    """
    from concourse import bacc
    import concourse.mybir as mybir
    import concourse.tile as tile

    f32 = mybir.dt.float32
    f8 = mybir.dt.float8e4
    odt = getattr(mybir.dt, out_dt)

    assert B % P == 0 and D % P == 0 and U % pass_w == 0 and pass_w % 512 == 0
    MT = B // P            # m-tiles (8)
    KT = D // P            # k-subtiles (16)
    NQ = U // pass_w       # n passes (4)
    NB = pass_w // 512     # psum banks per (m-tile, pass) (1)
    KS = KT // wsub        # w sub-tiles per pass (4)
    XG = MT // xgrp        # x^T groups (4)
    step = 2 if use_dr else 1
    assert wsub % step == 0

    nc = bacc.Bacc("TRN2", target_bir_lowering=False)
    x_d = nc.dram_tensor("xT", [D, B], f32, kind="ExternalInput")
    w_d = nc.dram_tensor("w", [D, U], f32, kind="ExternalInput")
    o_d = nc.dram_tensor("out", [B, U], odt, kind="ExternalOutput")

    x_ap = x_d[:].rearrange("(s p) m -> p s m", p=P)       # [128, KT, B]
    w_ap = w_d[:].rearrange("(s p) u -> p s u", p=P)       # [128, KT, U]
    o_ap = o_d[:].rearrange("(j p) u -> j p u", p=P)       # [MT, 128, U]

    GE = mybir.AluOpType.is_ge
    SUB = mybir.AluOpType.subtract

    with tile.TileContext(nc) as tc, \
         tc.tile_pool(name="wstage", bufs=3) as wstage, \
         tc.tile_pool(name="xstage", bufs=2) as xstage, \
         tc.tile_pool(name="resident", bufs=1) as resident, \
         tc.tile_pool(name="mpsum", bufs=8, space="PSUM") as mpsum:

        # separate resident tiles => fine-grained scheduler dependencies
        w8 = [[resident.tile([P, wsub, pass_w], f8, name=f"w8_{q}_{h}")
               for h in range(KS)] for q in range(NQ)]
        xT8 = [resident.tile([P, KT, xgrp * P], f8, name=f"xT8_{g}")
               for g in range(XG)]
        ost = [resident.tile([P, U], odt, name=f"ost_{j}")
               for j in range(MT)]

        def emit_x_group(g):
            m0 = g * xgrp * P
            xs = xstage.tile([P, KT, xgrp * P], f32, tag="xs")
            nc.sync.dma_start(xs, x_ap[:, :, m0:m0 + xgrp * P])
            nc.vector.tensor_scalar(xT8[g], xs, 0.0, 0.5, GE, SUB)

        def emit_w_subtile(q, h):
            n0 = q * pass_w
            ws = wstage.tile([P, wsub, pass_w], f32, tag="ws")
            ks0 = h * wsub
            nc.sync.dma_start(ws, w_ap[:, ks0:ks0 + wsub, n0:n0 + pass_w])
            nc.vector.tensor_scalar(w8[q][h], ws, 0.0, 0.5, GE, SUB)

        def emit_mm(q, j):
            g, jo = j // xgrp, (j % xgrp) * P
            pss = [mpsum.tile([P, 512], f32, tag="ps", name=f"ps_{q}_{j}_{b}")
                   for b in range(NB)]
            for h in range(KS):
                for kc in range(0, wsub, step):
                    ks = h * wsub + kc
                    first = ks == 0
                    last = ks + step >= KT
                    for b in range(NB):
                        if use_dr:
                            nc.tensor.matmul(
                                pss[b],
                                lhsT=xT8[g][:, ks:ks + 2, jo:jo + P],
                                rhs=w8[q][h][:, kc:kc + 2, 512 * b:512 * (b + 1)],
                                start=first, stop=last,
                                perf_mode=mybir.MatmulPerfMode.DoubleRow,
                            )
                        else:
                            nc.tensor.matmul(
                                pss[b],
                                lhsT=xT8[g][:, ks, jo:jo + P],
                                rhs=w8[q][h][:, kc, 512 * b:512 * (b + 1)],
                                start=first, stop=last,
                            )
            for b in range(NB):
                # evict with x4 scale: (+-0.5 * +-0.5) sums -> integer out
                nc.scalar.activation(
                    ost[j][:, q * pass_w + 512 * b:q * pass_w + 512 * (b + 1)],
                    pss[b], mybir.ActivationFunctionType.Copy, scale=4.0,
                )

        def emit_store(j, q=None):
            if q is None:
                nc.sync.dma_start(o_ap[j], ost[j])
            else:
                n0 = q * pass_w
                nc.sync.dma_start(
                    o_ap[j, :, n0:n0 + pass_w], ost[j][:, n0:n0 + pass_w])

        # Emission schedule (defines DMA queue order and per-engine
        # instruction order). Interleave w pass-slices, x groups and
        # matmuls so PE work is available as data lands.
        if (NQ, KS, XG, MT) == (4, 4, 4, 8):
            emit_w_subtile(0, 0)
            emit_x_group(0)
            emit_w_subtile(0, 1)
            emit_w_subtile(0, 2)
            emit_w_subtile(0, 3)
            emit_mm(0, 0)
            emit_mm(0, 1)
            emit_x_group(1)
            emit_mm(0, 2)
            emit_mm(0, 3)
            emit_x_group(2)
            emit_w_subtile(1, 0)
            emit_w_subtile(1, 1)
            emit_mm(0, 4)
            emit_mm(0, 5)
            emit_w_subtile(1, 2)
            emit_w_subtile(1, 3)
            emit_x_group(3)
            emit_mm(1, 0)
            emit_mm(1, 1)
            emit_mm(1, 2)
            emit_mm(1, 3)
            emit_mm(0, 6)
            emit_mm(0, 7)
            for h in range(KS):
                emit_w_subtile(2, h)
            for j in range(4, MT):
                emit_mm(1, j)
            for j in range(0, 4):
                emit_mm(2, j)
            for h in range(KS):
                emit_w_subtile(3, h)
            for j in range(4, MT):
                emit_mm(2, j)
            # early stores: land in the DMA-idle window after the last
            # input DMA, while pass-2/3 matmuls still run
            for j in range(MT):
                emit_store(j, 0)
            for j in range(MT):
                emit_store(j, 1)
            for j in range(MT):
                emit_mm(3, j)
                emit_store(j, 2)
                emit_store(j, 3)
        else:
            # generic fallback (used by small-shape tests)
            for q in range(NQ):
                for h in range(KS):
                    emit_w_subtile(q, h)
            for g in range(XG):
                emit_x_group(g)
            for q in range(NQ):
                for j in range(MT):
                    emit_mm(q, j)
                    if q == NQ - 1:
                        emit_store(j)

    nc.compile()
    return nc


_NC_CACHE = {}
LAST_RESULTS = {}


def _get_nc(**kwargs):
    key = tuple(sorted(kwargs.items()))
    if key not in _NC_CACHE:
        _NC_CACHE[key] = build_kernel(**kwargs)
    return _NC_CACHE[key]


def kernel(x, w, _trace=False, _trace_cores=None, **build_kwargs):
    from concourse.bass_utils import run_bass_kernel_spmd

    x = np.asarray(x, dtype=np.float32)
    w = np.asarray(w, dtype=np.float32)
    assert x.shape == (B_FULL, D_IN) and w.shape == (D_IN, UNITS)

    nc = _get_nc(**build_kwargs)
    in_maps = [
        {"xT": np.ascontiguousarray(x[c * B_CORE:(c + 1) * B_CORE].T),
         "w": w}
        for c in range(N_CORES)
    ]
    br = run_bass_kernel_spmd(
        nc, in_maps, list(range(N_CORES)),
        trace=_trace, trace_cores=_trace_cores,
    )
    LAST_RESULTS["br"] = br
    out = np.concatenate(
        [br.results[c]["out"].astype(np.float32) for c in range(N_CORES)],
        axis=0,
    )
    return out


if __name__ == "__main__":
    rng = np.random.default_rng(0)
    x = rng.standard_normal((B_FULL, D_IN), dtype=np.float32)
    w = (rng.standard_normal((D_IN, UNITS), dtype=np.float32) * 0.1).astype(
        np.float32
    )
    out = kernel(x, w)
    exp = np.sign(x + (x == 0)) @ np.sign(w + (w == 0))
    print("max abs err:", np.max(np.abs(out - exp)))


# revision 9
# speedup vs baseline: 1.3730x; 1.0068x over previous
"""BinaryDense kernel for Trainium2: out = sign(x) @ sign(w).

x: [8192, 2048] f32, w: [2048, 2048] f32 -> out: [8192, 2048] f32.

Strategy: data-parallel shard of the batch dim across 8 NeuronCores
(1024 rows each, w replicated). The host hands each core its x shard
pre-transposed (pure layout choice for the shard), so the contraction
dim lands on SBUF partitions with no on-chip transpose. Per core:
  - w streamed in (pass, k-sub-tile) slices of [128, 4ksub, 512cols]
    (1MB granule), binarized on DVE to fp8e4 (+-0.5 via one
    tensor_scalar: (w >= 0) - 0.5) into separate small resident tiles
    (separate tiles => fine-grained scheduler dependencies, so matmuls
    start as soon as their slice has landed).
  - x^T loaded in m-pair column groups [128, 16ksub, 256], binarized the
    same way into fp8 residents.
  - Matmuls in fp8 DoubleRow mode (K=256 per instruction, N=512),
    accumulating in PSUM fp32, n-pass-major (4 passes of 512 columns) so
    compute overlaps the w DMA stream.
  - PSUM evicted on the Scalar engine with scale=4.0 (products are
    (+-0.5)^2 = +-0.25) to fp16 output tiles (integers <= 2048 are
    exact in fp16), halving the store traffic; host widens to f32.

All arithmetic is exact: +-0.5 exact in fp8e4, products +-0.25 exact,
sums are multiples of 0.25 bounded by 512 (fp32-exact), x4 exact,
results are integers in [-2048, 2048], all exactly representable in
fp16. The host fp16->f32 widening is exact.
"""

import sys

if "/opt/trn_rl_repo" not in sys.path:
    sys.path.insert(0, "/opt/trn_rl_repo")

import numpy as np

B_FULL, D_IN, UNITS = 8192, 2048, 2048
N_CORES = 8
B_CORE = B_FULL // N_CORES  # 1024
P = 128


def build_kernel(B=B_CORE, D=D_IN, U=UNITS, pass_w=512, wsub=4, xgrp=2,
                 use_dr=True, out_dt="float16"):
    """Build (and compile) the per-core Bass kernel. Returns the Bacc nc.

    pass_w: output columns per pass (multiple of 512)
    wsub:   k-subtiles per resident w sub-tile (DMA/binarize granule)
    xgrp:   m-tiles (128 cols of x^T) per resident x^T group
    """
    from concourse import bacc
    import concourse.mybir as mybir
    import concourse.tile as tile

    f32 = mybir.dt.float32
    f8 = mybir.dt.float8e4
    odt = getattr(mybir.dt, out_dt)

    assert B % P == 0 and D % P == 0 and U % pass_w == 0 and pass_w % 512 == 0
    MT = B // P            # m-tiles (8)
    KT = D // P            # k-subtiles (16)
    NQ = U // pass_w       # n passes (4)
    NB = pass_w // 512     # psum banks per (m-tile, pass) (1)
    KS = KT // wsub        # w sub-tiles per pass (4)
    XG = MT // xgrp        # x^T groups (4)
    step = 2 if use_dr else 1
    assert wsub % step == 0

    nc = bacc.Bacc("TRN2", target_bir_lowering=False)
    x_d = nc.dram_tensor("xT", [D, B], f32, kind="ExternalInput")
    w_d = nc.dram_tensor("w", [D, U], f32, kind="ExternalInput")
    o_d = nc.dram_tensor("out", [B, U], odt, kind="ExternalOutput")

    x_ap = x_d[:].rearrange("(s p) m -> p s m", p=P)       # [128, KT, B]
    w_ap = w_d[:].rearrange("(s p) u -> p s u", p=P)       # [128, KT, U]
    o_ap = o_d[:].rearrange("(j p) u -> j p u", p=P)       # [MT, 128, U]

    GE = mybir.AluOpType.is_ge
    SUB = mybir.AluOpType.subtract

    with tile.TileContext(nc) as tc, \
         tc.tile_pool(name="wstage", bufs=3) as wstage, \
         tc.tile_pool(name="xstage", bufs=2) as xstage, \
         tc.tile_pool(name="resident", bufs=1) as resident, \
         tc.tile_pool(name="mpsum", bufs=8, space="PSUM") as mpsum:

        # separate resident tiles => fine-grained scheduler dependencies
        w8 = [[resident.tile([P, wsub, pass_w], f8, name=f"w8_{q}_{h}")
               for h in range(KS)] for q in range(NQ)]
        xT8 = [resident.tile([P, KT, xgrp * P], f8, name=f"xT8_{g}")
               for g in range(XG)]
        ost = [resident.tile([P, U], odt, name=f"ost_{j}")
               for j in range(MT)]

        def emit_x_group(g):
            m0 = g * xgrp * P
            xs = xstage.tile([P, KT, xgrp * P], f32, tag="xs")
            nc.sync.dma_start(xs, x_ap[:, :, m0:m0 + xgrp * P])
            nc.vector.tensor_scalar(xT8[g], xs, 0.0, 0.5, GE, SUB)

        def emit_w_subtile(q, h):
            n0 = q * pass_w
            ws = wstage.tile([P, wsub, pass_w], f32, tag="ws")
            ks0 = h * wsub
            nc.sync.dma_start(ws, w_ap[:, ks0:ks0 + wsub, n0:n0 + pass_w])
            nc.vector.tensor_scalar(w8[q][h], ws, 0.0, 0.5, GE, SUB)

        psum_tiles = {}

        def emit_mm_chunk(q, j, h):
            g, jo = j // xgrp, (j % xgrp) * P
            if (q, j) not in psum_tiles:
                psum_tiles[(q, j)] = [
                    mpsum.tile([P, 512], f32, tag="ps", name=f"ps_{q}_{j}_{b}")
                    for b in range(NB)]
            pss = psum_tiles[(q, j)]
            for kc in range(0, wsub, step):
                ks = h * wsub + kc
                first = ks == 0
                last = ks + step >= KT
                for b in range(NB):
                    if use_dr:
                        nc.tensor.matmul(
                            pss[b],
                            lhsT=xT8[g][:, ks:ks + 2, jo:jo + P],
                            rhs=w8[q][h][:, kc:kc + 2, 512 * b:512 * (b + 1)],
                            start=first, stop=last,
                            perf_mode=mybir.MatmulPerfMode.DoubleRow,
                        )
                    else:
                        nc.tensor.matmul(
                            pss[b],
                            lhsT=xT8[g][:, ks, jo:jo + P],
                            rhs=w8[q][h][:, kc, 512 * b:512 * (b + 1)],
                            start=first, stop=last,
                        )

        def emit_evict(q, j):
            pss = psum_tiles.pop((q, j))
            for b in range(NB):
                # evict with x4 scale: (+-0.5 * +-0.5) sums -> integer out
                nc.scalar.activation(
                    ost[j][:, q * pass_w + 512 * b:q * pass_w + 512 * (b + 1)],
                    pss[b], mybir.ActivationFunctionType.Copy, scale=4.0,
                )

        def emit_mm(q, j):
            for h in range(KS):
                emit_mm_chunk(q, j, h)
            emit_evict(q, j)

        def emit_store(j, q=None):
            if q is None:
                nc.sync.dma_start(o_ap[j], ost[j])
            else:
                n0 = q * pass_w
                nc.sync.dma_start(
                    o_ap[j, :, n0:n0 + pass_w], ost[j][:, n0:n0 + pass_w])

        # Emission schedule (defines DMA queue order and per-engine
        # instruction order). Interleave w pass-slices, x groups and
        # matmuls so PE work is available as data lands.
        if (NQ, KS, XG, MT) == (4, 4, 4, 8):
            # DMA stream order (1-2MB granules, ~2.9us/MB): w quarter q
            # just-in-time for pass q, x groups woven in for pass 0.
            emit_w_subtile(0, 0)
            emit_x_group(0)              # m0, m1
            emit_w_subtile(0, 1)
            emit_w_subtile(0, 2)
            # pass 0 j-major (x-arrival paced)
            emit_mm_chunk(0, 0, 0)
            emit_mm_chunk(0, 1, 0)
            emit_x_group(1)              # m2, m3
            emit_mm_chunk(0, 0, 1)
            emit_mm_chunk(0, 1, 1)
            emit_w_subtile(0, 3)
            emit_mm_chunk(0, 0, 2)
            emit_mm_chunk(0, 1, 2)
            emit_x_group(2)              # m4, m5
            emit_mm(0, 2)
            emit_mm_chunk(0, 0, 3)
            emit_evict(0, 0)
            emit_mm_chunk(0, 1, 3)
            emit_evict(0, 1)
            emit_w_subtile(1, 0)
            emit_mm(0, 3)
            emit_w_subtile(1, 1)
            emit_mm(0, 4)
            emit_x_group(3)              # m6, m7
            emit_mm(0, 5)
            emit_w_subtile(1, 2)
            emit_w_subtile(1, 3)
            # pass 1 h-major for j0..5 (w-arrival paced), weave in m6/m7
            for j in range(6):
                emit_mm_chunk(1, j, 0)
            for j in range(6):
                emit_mm_chunk(1, j, 1)
            emit_mm(0, 6)
            for j in range(6):
                emit_mm_chunk(1, j, 2)
            emit_mm(0, 7)
            for j in range(6):
                emit_mm_chunk(1, j, 3)
                emit_evict(1, j)
            for h in range(KS):
                emit_w_subtile(2, h)
            emit_mm(1, 6)
            emit_mm(1, 7)
            # pass 2 h-major, all j
            for h in range(KS):
                for j in range(MT):
                    emit_mm_chunk(2, j, h)
                    if h == KS - 1:
                        emit_evict(2, j)
            for h in range(KS):
                emit_w_subtile(3, h)
            # early stores land in the post-input DMA window
            for j in range(MT):
                emit_store(j, 0)
            for j in range(MT):
                emit_store(j, 1)
            # pass 3 h-major, all j
            for h in range(KS):
                for j in range(MT):
                    emit_mm_chunk(3, j, h)
                    if h == KS - 1:
                        emit_evict(3, j)
                        emit_store(j, 3)
            for j in range(MT):
                emit_store(j, 2)
        else:
            # generic fallback (used by small-shape tests)
            for q in range(NQ):
                for h in range(KS):
                    emit_w_subtile(q, h)
            for g in range(XG):
                emit_x_group(g)
            for q in range(NQ):
                for j in range(MT):
                    emit_mm(q, j)
                    if q == NQ - 1:
                        emit_store(j)

    nc.compile()
    return nc


_NC_CACHE = {}
LAST_RESULTS = {}


def _get_nc(**kwargs):
    key = tuple(sorted(kwargs.items()))
    if key not in _NC_CACHE:
        _NC_CACHE[key] = build_kernel(**kwargs)
    return _NC_CACHE[key]


def kernel(x, w, _trace=False, _trace_cores=None, **build_kwargs):
    from concourse.bass_utils import run_bass_kernel_spmd

    x = np.asarray(x, dtype=np.float32)
    w = np.asarray(w, dtype=np.float32)
    assert x.shape == (B_FULL, D_IN) and w.shape == (D_IN, UNITS)

    nc = _get_nc(**build_kwargs)
    in_maps = [
        {"xT": np.ascontiguousarray(x[c * B_CORE:(c + 1) * B_CORE].T),
         "w": w}
        for c in range(N_CORES)
    ]
    br = run_bass_kernel_spmd(
        nc, in_maps, list(range(N_CORES)),
        trace=_trace, trace_cores=_trace_cores,
    )
    LAST_RESULTS["br"] = br
    out = np.concatenate(
        [br.results[c]["out"].astype(np.float32) for c in range(N_CORES)],
        axis=0,
    )
    return out


if __name__ == "__main__":
    rng = np.random.default_rng(0)
    x = rng.standard_normal((B_FULL, D_IN), dtype=np.float32)
    w = (rng.standard_normal((D_IN, UNITS), dtype=np.float32) * 0.1).astype(
        np.float32
    )
    out = kernel(x, w)
    exp = np.sign(x + (x == 0)) @ np.sign(w + (w == 0))
    print("max abs err:", np.max(np.abs(out - exp)))
